# revision 3
# baseline (speedup 1.0000x reference)
"""Trainium2 Bass kernel for DenseDilatedKnnGraph (B=4, C=128, N=8192, k=9, dilation=4).

Strategy
--------
Candidates are ranked by s = <xn, yn> (|xn|^2 per-row constant, |yn|^2 == 1).
Device computes, per query row, the top-8 of each of 16 candidate groups of
512 — but as a SINGLE DVE max8 pass over index-packed values, instead of the
2-pass max + max_index:

  PSUM accumulation chain per group (all on the PE, fp32r at 1 cyc/row):
    1. score matmul:  s*2048          (xn pre-scaled by 2048 on host)
    2. + 2^23        -> rounds bank to integer grid (fp32 RNE at PSUM add)
    3. - 2^23        -> q = round(s*2048), exact
    4. + (511-j)/4096 -> p = q + (511-j)/4096, exact fp32; j = in-group index

  One vector.max over the bank yields the top-8 packed (value, index) pairs.

Host decodes (q, j) from p, exact-refines the top-T candidates per row in
fp64, and recomputes rows exactly where a group's 8-candidate capacity (or
the refinement set) could hide a true top-33 member.

Sharding: 8 cores = 4 batches x 2 query-halves; each core gets its 4096 query
columns of xn[b]*2048 plus the full yn[b] (both channel-major [128, N]).
"""

import os
import numpy as np

import concourse.bacc as bacc
import concourse.mybir as mybir
from concourse.tile import TileContext
from concourse.bass_utils import run_bass_kernel_spmd

# problem constants (hardcoded per harness contract)
B, C, N = 4, 128, 8192
K_OUT, DIL = 9, 4
KK = K_OUT * DIL            # 36
NQ = N // 2                 # 4096 query rows per core
TILES = NQ // 128           # 32
GS = 512                    # candidate group size == PSUM bank
G = N // GS                 # 16 groups
EPS = 1e-12
F32 = mybir.dt.float32
F32R = mybir.dt.float32r
MAGIC = float(2.0 ** 23)
QSCALE = 2048.0             # score quantization: q = round(s * 2048)
T_REF = 80                  # host-refined candidates per row
FP32R_MARGIN = 3.0          # quanta; device-q vs exact-score slack

_CACHED = {}


def _build():
    nc = bacc.Bacc("TRN2")
    xs = nc.dram_tensor("xs", [C, NQ], F32R, kind="ExternalInput")   # xn*2048
    yf = nc.dram_tensor("yf", [C, N], F32R, kind="ExternalInput")
    cs = nc.dram_tensor("cs", [3, GS], F32R, kind="ExternalInput")   # magic,unmagic,iota
    on = nc.dram_tensor("on", [1, 128], F32R, kind="ExternalInput")  # ones lhsT
    o_p = nc.dram_tensor("o_p", [TILES, 128, G * 8], F32, kind="ExternalOutput")

    with TileContext(nc) as tc:
        with (
            tc.tile_pool(name="persist", bufs=1) as persist,
            tc.tile_pool(name="cpool", bufs=4) as cpool,
            tc.tile_pool(name="mpsum", bufs=8, space="PSUM") as mpsum,
        ):
            yn = persist.tile([C, N], F32R, tag="yn")
            xn = persist.tile([C, NQ], F32R, tag="xn")
            mg = persist.tile([1, GS], F32R, tag="mg")
            um = persist.tile([1, GS], F32R, tag="um")
            io = persist.tile([1, GS], F32R, tag="io")
            ones = persist.tile([1, 128], F32R, tag="ones")
            nc.sync.dma_start(mg, cs[0:1, :])
            nc.sync.dma_start(um, cs[1:2, :])
            nc.sync.dma_start(io, cs[2:3, :])
            nc.sync.dma_start(ones, on[:, :])
            # chunked loads so tile 0's matmuls start after the first chunks
            nc.sync.dma_start(xn[:, :GS], xs[:, :GS])
            for j in range(G):
                sl = slice(j * GS, (j + 1) * GS)
                nc.sync.dma_start(yn[:, sl], yf[:, sl])
            for j in range(1, NQ // GS):
                sl = slice(j * GS, (j + 1) * GS)
                nc.sync.dma_start(xn[:, sl], xs[:, sl])

            for t in range(TILES):
                lhsT = xn[:, t * 128:(t + 1) * 128]
                Ct = cpool.tile([128, G * 8], F32, tag="C")
                # waves of 4 groups: batch score-mms (shared weights), then
                # the packing accumulates (shared ones weights), then max8
                for w in range(G // 4):
                    banks = []
                    for i in range(4):
                        g = w * 4 + i
                        sl = slice(g * GS, (g + 1) * GS)
                        ps = mpsum.tile([128, GS], F32, tag="ps")
                        nc.tensor.matmul(ps, lhsT, yn[:, sl],
                                         start=True, stop=False)
                        banks.append(ps)
                    for ps in banks:
                        nc.tensor.matmul(ps, ones, mg, start=False, stop=False)
                        nc.tensor.matmul(ps, ones, um, start=False, stop=False)
                        nc.tensor.matmul(ps, ones, io, start=False, stop=True)
                    for i, ps in enumerate(banks):
                        g = w * 4 + i
                        nc.vector.max(Ct[:, 8 * g:8 * g + 8], ps)

                nc.sync.dma_start(o_p[t, :, :], Ct)
    nc.finalize()
    return nc


def _host_normalize(t):
    # mimics reference._l2_normalize over axis 0 of a [C, N] f32 array
    n = np.sqrt(np.sum(t * t, axis=0, keepdims=True, dtype=np.float32),
                dtype=np.float32)
    return (t / np.maximum(n, np.float32(EPS))).astype(np.float32)


def kernel(x, y):
    x = np.ascontiguousarray(np.asarray(x, dtype=np.float32)[..., 0])  # (B, C, N)
    y = np.ascontiguousarray(np.asarray(y, dtype=np.float32)[..., 0])

    xn = np.stack([_host_normalize(x[b]) for b in range(B)])
    yn = np.stack([_host_normalize(y[b]) for b in range(B)])

    if "nc" not in _CACHED:
        _CACHED["nc"] = _build()
    nc = _CACHED["nc"]

    iota = (511.0 - np.arange(GS, dtype=np.float64)) / 4096.0
    cs = np.stack([np.full(GS, MAGIC, np.float32),
                   np.full(GS, -MAGIC, np.float32),
                   iota.astype(np.float32)])
    on = np.ones((1, 128), np.float32)

    in_maps = []
    for k in range(8):
        b, h = k // 2, k % 2
        in_maps.append({
            "xs": np.ascontiguousarray(
                xn[b, :, h * NQ:(h + 1) * NQ] * np.float32(QSCALE)),
            "yf": yn[b],
            "cs": cs,
            "on": on,
        })

    trace = bool(int(os.environ.get("KNN_TRACE", "0")))
    res = run_bass_kernel_spmd(nc, in_maps, core_ids=list(range(8)), trace=trace)
    if res.exec_time_ns is not None:
        print(f"HW exec time: {res.exec_time_ns} ns")
        _CACHED["exec_time_ns"] = res.exec_time_ns

    nn_idx = np.zeros((B, N, KK), np.int32)
    need_fallback = []
    qerr_max = 0.0
    for k in range(8):
        b, h = k // 2, k % 2
        pv = res.results[k]["o_p"].reshape(NQ, G * 8).astype(np.float64)
        v = np.rint(pv * 4096.0).astype(np.int64)
        qv = v >> 12                                   # round(s*2048)
        jj = 511 - (v & 4095)                          # in-group index
        slot_g = np.arange(G * 8, dtype=np.int64) >> 3
        idx = (slot_g[None, :] * GS + jj).astype(np.int64)   # [NQ, 128] original idx

        # top-T by packed value (q desc, then idx asc)
        order = np.argsort(-pv, axis=1, kind="stable")[:, :T_REF]
        rows = np.arange(NQ)[:, None]
        cand = idx[rows, order]                        # [NQ, T]
        # exact scores in fp64
        ynbT = np.ascontiguousarray(yn[b].T)           # [N, C]
        xh = xn[b][:, h * NQ:(h + 1) * NQ]             # [C, NQ]
        gsel = ynbT[cand].astype(np.float64)           # [NQ, T, C]
        s_ex = np.matmul(gsel, xh.T.astype(np.float64)[:, :, None])[..., 0]  # [NQ, T]

        # diagnostic: device-q vs exact score (quanta)
        qerr_max = max(qerr_max, float(
            np.abs(s_ex * QSCALE - qv[rows, order]).max()))

        # exact stable top-KK among refined: (-s, idx) lexicographic
        sel = np.lexsort((cand, -s_ex), axis=1)[:, :KK]
        top_idx = np.take_along_axis(cand, sel, axis=1)
        top_s = np.take_along_axis(s_ex, sel, axis=1)
        nn_idx[b, h * NQ:(h + 1) * NQ, :] = top_idx

        # safety: a candidate outside the kept sets could still make top-33.
        # any unshipped group member h has q(h) <= q8_g (group's 8th kept),
        # so exact(h) <= (q8_g + 0.5 + fp32r_margin)/2048; same bound with
        # q at slot T for candidates shipped but not refined.
        cutoff = top_s[:, 32]                          # 33rd exact (rank 32)
        bound = (np.float64(0.5) + FP32R_MARGIN) / QSCALE
        q8 = qv[:, 7::8].astype(np.float64) / QSCALE   # [NQ, G]
        risk_g = ((q8 + bound) >= cutoff[:, None]).any(axis=1)
        qT = np.sort(qv, axis=1)[:, ::-1][:, T_REF].astype(np.float64) / QSCALE
        risk_T = (qT + bound) >= cutoff
        risk = risk_g | risk_T | (np.abs(qv) >= 2047).any(axis=1)
        for r in np.nonzero(risk)[0]:
            need_fallback.append((b, h * NQ + int(r)))

    if need_fallback:
        by_batch = {}
        for b, n_ in need_fallback:
            by_batch.setdefault(b, []).append(n_)
        for b, rows_ in by_batch.items():
            ynb = yn[b].astype(np.float64)                    # (C, N)
            xnr = xn[b][:, rows_].astype(np.float64)          # (C, R)
            s = xnr.T @ ynb                                   # (R, N)
            part = np.argpartition(-s, KK + 8, axis=1)[:, :KK + 8]
            rr = np.arange(len(rows_))[:, None]
            pvx = -s[rr, part]
            order = np.lexsort((part, pvx), axis=1)[:, :KK]
            top = np.take_along_axis(part, order, axis=1)
            nn_idx[b, rows_, :] = top

    _CACHED["fallback_rows"] = len(need_fallback)
    _CACHED["qerr_max"] = qerr_max

    center = np.broadcast_to(np.arange(N, dtype=np.int32)[None, :, None],
                             (B, N, K_OUT))
    edge = np.stack([np.ascontiguousarray(nn_idx[:, :, ::DIL]), center], axis=0)
    return edge.astype(np.int32)


# revision 13
# speedup vs baseline: 5.5838x; 5.5838x over previous
"""Trainium2 Bass kernel for DenseDilatedKnnGraph (B=4, C=128, N=8192, k=9, dilation=4).

Strategy
--------
Candidates are ranked by s = <xn, yn>. The expensive part of the baseline was
two full DVE passes (max8 + max_index) over all 33.5M scores per core. Here
the scan is collapsed 4:1 before the DVE top-8 passes, spreading work over
four engines:

  per 2048-candidate quarter of a 128-query tile:
    PE : 4 bf16 score matmuls -> one 4-bank PSUM tile  (s in fp32)
    Act: one copy PSUM -> SBUF bf16 (S16, 2048 cols)
    GpSimd: elementwise max tree: M4[j] = max(S16[j], S16[j+512],
            S16[j+1024], S16[j+1536])      (3 tensor_max ops, 512 cols each)
    DVE: per 128-slot group: max8 -> top-8 slot values (bf16),
         max_index -> slot positions.      (scans 512 slots, not 2048 cols)

Each kept slot is ambiguous over its 4 source columns; the host exact-refines
all 4 members of the top slots per row in fp64, so ambiguity is free. Rows
where the 8-slots-per-group capacity (or the refinement set) could hide a
true top-33 member are recomputed exactly on host (~2-3% of rows).

Sharding: 8 cores = 4 batches x 2 query-halves; each core gets its 4096 query
columns of xn[b] plus the full yn[b] (channel-major, bf16).
"""

import os
import numpy as np
import ml_dtypes

import concourse.bacc as bacc
import concourse.mybir as mybir
from concourse.tile import TileContext
from concourse.bass_utils import run_bass_kernel_spmd

# problem constants (hardcoded per harness contract)
B, C, N = 4, 128, 8192
K_OUT, DIL = 9, 4
KK = K_OUT * DIL            # 36
NQ = N // 2                 # 4096 query rows per core
TILES = NQ // 128           # 32
GS = 512                    # candidate group size == PSUM bank
G = N // GS                 # 16 groups (of slot capacity 8)
QT = 2048                   # quarter-tile: 4 banks
EPS = 1e-12
F32 = mybir.dt.float32
BF16 = mybir.dt.bfloat16
U16 = mybir.dt.uint16
T_SLOTS = 44                # host-refined slots per row (x4 candidates)
EPS_MM = 4.0e-3             # |exact - bf16 matmul| score slack (abs)

_CACHED = {}


def _build():
    nc = bacc.Bacc("TRN2")
    xs = nc.dram_tensor("xs", [C, NQ], BF16, kind="ExternalInput")
    yf = nc.dram_tensor("yf", [C, N], BF16, kind="ExternalInput")
    o_v = nc.dram_tensor("o_v", [TILES, 128, G * 8], BF16, kind="ExternalOutput")
    o_i = nc.dram_tensor("o_i", [TILES, 128, G * 8], U16, kind="ExternalOutput")

    with TileContext(nc) as tc:
        with (
            tc.tile_pool(name="persist", bufs=1) as persist,
            tc.tile_pool(name="spool", bufs=3) as spool,
            tc.tile_pool(name="mpool", bufs=3) as mpool,
            tc.tile_pool(name="cpool", bufs=3) as cpool,
            tc.tile_pool(name="mpsum", bufs=4, space="PSUM") as mpsum,
        ):
            # separate tiles per chunk so the first matmuls depend only on
            # their own chunk's DMA, not the whole 3MB input load
            yc = [persist.tile([C, 1024], BF16, name=f"yc{j}", tag=f"yc{j}")
                  for j in range(N // 1024)]
            xc = [persist.tile([C, 512], BF16, name=f"xc{j}", tag=f"xc{j}")
                  for j in range(NQ // 512)]
            nc.sync.dma_start(xc[0], xs[:, :512])
            nc.sync.dma_start(yc[0], yf[:, :1024])
            nc.sync.dma_start(yc[1], yf[:, 1024:2048])
            for j in range(2, N // 1024):
                nc.sync.dma_start(yc[j], yf[:, j * 1024:(j + 1) * 1024])
            for j in range(1, NQ // 512):
                nc.sync.dma_start(xc[j], xs[:, j * 512:(j + 1) * 512])

            for t in range(TILES):
                lhsT = xc[t // 4][:, (t % 4) * 128:(t % 4 + 1) * 128]
                Vt = cpool.tile([128, G * 8], BF16, tag="V")
                It = cpool.tile([128, G * 8], U16, tag="I")
                for q in range(N // QT):
                    ps = mpsum.tile([128, QT], F32, tag="ps")
                    for i in range(4):
                        nc.tensor.matmul(
                            ps[:, i * GS:(i + 1) * GS], lhsT,
                            yn[:, q * QT + i * GS: q * QT + (i + 1) * GS],
                            start=True, stop=True)
                    S16 = spool.tile([128, QT], BF16, tag="S16")
                    nc.scalar.copy(S16, ps)
                    T1 = mpool.tile([128, GS], BF16, tag="T1")
                    T2 = mpool.tile([128, GS], BF16, tag="T2")
                    M4 = mpool.tile([128, GS], BF16, tag="M4")
                    nc.vector.tensor_max(T1, S16[:, 0:GS], S16[:, GS:2 * GS])
                    nc.vector.tensor_max(T2, S16[:, 2 * GS:3 * GS],
                                         S16[:, 3 * GS:4 * GS])
                    nc.vector.tensor_max(M4, T1, T2)
                    for k in range(4):
                        g = 4 * q + k
                        m4k = M4[:, k * 128:(k + 1) * 128]
                        nc.vector.max(Vt[:, 8 * g:8 * g + 8], m4k)
                        nc.vector.max_index(It[:, 8 * g:8 * g + 8],
                                            Vt[:, 8 * g:8 * g + 8], m4k)

                nc.sync.dma_start(o_v[t, :, :], Vt)
                nc.sync.dma_start(o_i[t, :, :], It)
    nc.finalize()
    return nc


def _host_normalize(t):
    # mimics reference._l2_normalize over axis 0 of a [C, N] f32 array
    n = np.sqrt(np.sum(t * t, axis=0, keepdims=True, dtype=np.float32),
                dtype=np.float32)
    return (t / np.maximum(n, np.float32(EPS))).astype(np.float32)


def kernel(x, y):
    x = np.ascontiguousarray(np.asarray(x, dtype=np.float32)[..., 0])  # (B, C, N)
    y = np.ascontiguousarray(np.asarray(y, dtype=np.float32)[..., 0])

    xn = np.stack([_host_normalize(x[b]) for b in range(B)])
    yn = np.stack([_host_normalize(y[b]) for b in range(B)])

    if "nc" not in _CACHED:
        _CACHED["nc"] = _build()
    nc = _CACHED["nc"]

    in_maps = []
    for k in range(8):
        b, h = k // 2, k % 2
        in_maps.append({
            "xs": np.ascontiguousarray(
                xn[b, :, h * NQ:(h + 1) * NQ]).astype(ml_dtypes.bfloat16),
            "yf": yn[b].astype(ml_dtypes.bfloat16),
        })

    trace = bool(int(os.environ.get("KNN_TRACE", "0")))
    res = run_bass_kernel_spmd(nc, in_maps, core_ids=list(range(8)), trace=trace)
    if res.exec_time_ns is not None:
        print(f"HW exec time: {res.exec_time_ns} ns")
        _CACHED["exec_time_ns"] = res.exec_time_ns

    nn_idx = np.zeros((B, N, KK), np.int32)
    need_fallback = []
    diag_max = 0.0
    # slot layout: kept slot (g, m): q = g//4, k = g%4, j = 128*k + m,
    # original candidates = 2048*q + j + 512*i, i in 0..3
    slot_g = np.arange(G * 8, dtype=np.int64) >> 3              # [128]
    slot_qbase = (slot_g // 4) * QT
    slot_kbase = (slot_g % 4) * 128
    for kcore in range(8):
        b, h = kcore // 2, kcore % 2
        vv = res.results[kcore]["o_v"].reshape(NQ, G * 8).astype(np.float64)
        mm = res.results[kcore]["o_i"].reshape(NQ, G * 8).astype(np.int64)
        j = slot_kbase[None, :] + mm                            # [NQ, 128]
        base = slot_qbase[None, :] + j                          # [NQ, 128]

        # top T_SLOTS by slot value
        order = np.argsort(-vv, axis=1, kind="stable")[:, :T_SLOTS]
        rows = np.arange(NQ)[:, None]
        cbase = base[rows, order]                               # [NQ, T]
        cand = (cbase[:, :, None] +
                np.arange(0, 4 * GS, GS)[None, None, :]).reshape(NQ, -1)

        # exact scores fp64, in two row-chunks to bound memory
        ynbT = np.ascontiguousarray(yn[b].T)                    # [N, C]
        xh = xn[b][:, h * NQ:(h + 1) * NQ]                      # [C, NQ]
        s_ex = np.empty(cand.shape, np.float64)
        for lo in range(0, NQ, 1024):
            hi = lo + 1024
            gsel = ynbT[cand[lo:hi]].astype(np.float64)         # [ch, 4T, C]
            s_ex[lo:hi] = np.matmul(
                gsel, xh.T[lo:hi].astype(np.float64)[:, :, None])[..., 0]

        # exact stable top-KK among refined
        sel = np.lexsort((cand, -s_ex), axis=1)[:, :KK]
        top_idx = np.take_along_axis(cand, sel, axis=1)
        top_s = np.take_along_axis(s_ex, sel, axis=1)
        nn_idx[b, h * NQ:(h + 1) * NQ, :] = top_idx

        # diagnostic: how far can exact scores sit above the kept slot value
        vmax = np.take_along_axis(vv, order, axis=1)            # [NQ, T]
        diag_max = max(diag_max, float(
            (s_ex - np.repeat(vmax, 2, axis=1)).max()))

        # safety: hidden candidate h has bf16(s_h) <= v8_g (its group's 8th
        # kept slot value) or <= v_(T+1) (slot kept but not refined), so
        # exact(h) <= v*(1+2^-8) + EPS_MM. flag row if that can reach the
        # 33rd exact score.
        cutoff = top_s[:, 32]
        v8 = vv[:, 7::8]                                        # [NQ, G]
        bnd_g = v8 + np.abs(v8) * (2.0 ** -8) + EPS_MM
        risk_g = (bnd_g >= cutoff[:, None]).any(axis=1)
        vT = np.sort(vv, axis=1)[:, ::-1][:, T_SLOTS]
        risk_T = (vT + np.abs(vT) * (2.0 ** -8) + EPS_MM) >= cutoff
        risk = risk_g | risk_T
        for r in np.nonzero(risk)[0]:
            need_fallback.append((b, h * NQ + int(r)))

    if need_fallback:
        by_batch = {}
        for b, n_ in need_fallback:
            by_batch.setdefault(b, []).append(n_)
        for b, rows_ in by_batch.items():
            ynb = yn[b].astype(np.float64)                      # (C, N)
            xnr = xn[b][:, rows_].astype(np.float64)            # (C, R)
            s = xnr.T @ ynb                                     # (R, N)
            part = np.argpartition(-s, KK + 8, axis=1)[:, :KK + 8]
            rr = np.arange(len(rows_))[:, None]
            pvx = -s[rr, part]
            order = np.lexsort((part, pvx), axis=1)[:, :KK]
            top = np.take_along_axis(part, order, axis=1)
            nn_idx[b, rows_, :] = top

    _CACHED["fallback_rows"] = len(need_fallback)
    _CACHED["diag_max"] = diag_max

    center = np.broadcast_to(np.arange(N, dtype=np.int32)[None, :, None],
                             (B, N, K_OUT))
    edge = np.stack([np.ascontiguousarray(nn_idx[:, :, ::DIL]), center], axis=0)
    return edge.astype(np.int32)


# revision 14
# speedup vs baseline: 5.6000x; 1.0029x over previous
"""Trainium2 Bass kernel for DenseDilatedKnnGraph (B=4, C=128, N=8192, k=9, dilation=4).

Strategy
--------
Candidates are ranked by s = <xn, yn>. The expensive part of the baseline was
two full DVE passes (max8 + max_index) over all 33.5M scores per core. Here
the scan is collapsed 4:1 before the DVE top-8 passes, spreading work over
four engines:

  per 2048-candidate quarter of a 128-query tile:
    PE : 4 bf16 score matmuls -> one 4-bank PSUM tile  (s in fp32)
    Act: one copy PSUM -> SBUF bf16 (S16, 2048 cols)
    GpSimd: elementwise max tree: M4[j] = max(S16[j], S16[j+512],
            S16[j+1024], S16[j+1536])      (3 tensor_max ops, 512 cols each)
    DVE: per 128-slot group: max8 -> top-8 slot values (bf16),
         max_index -> slot positions.      (scans 512 slots, not 2048 cols)

Each kept slot is ambiguous over its 4 source columns; the host exact-refines
all 4 members of the top slots per row in fp64, so ambiguity is free. Rows
where the 8-slots-per-group capacity (or the refinement set) could hide a
true top-33 member are recomputed exactly on host (~2-3% of rows).

Sharding: 8 cores = 4 batches x 2 query-halves; each core gets its 4096 query
columns of xn[b] plus the full yn[b] (channel-major, bf16).
"""

import os
import numpy as np
import ml_dtypes

import concourse.bacc as bacc
import concourse.mybir as mybir
from concourse.tile import TileContext
from concourse.bass_utils import run_bass_kernel_spmd

# problem constants (hardcoded per harness contract)
B, C, N = 4, 128, 8192
K_OUT, DIL = 9, 4
KK = K_OUT * DIL            # 36
NQ = N // 2                 # 4096 query rows per core
TILES = NQ // 128           # 32
GS = 512                    # candidate group size == PSUM bank
G = N // GS                 # 16 groups (of slot capacity 8)
QT = 2048                   # quarter-tile: 4 banks
EPS = 1e-12
F32 = mybir.dt.float32
BF16 = mybir.dt.bfloat16
U16 = mybir.dt.uint16
T_SLOTS = 44                # host-refined slots per row (x4 candidates)
EPS_MM = 4.0e-3             # |exact - bf16 matmul| score slack (abs)

_CACHED = {}


def _build():
    nc = bacc.Bacc("TRN2")
    xs = nc.dram_tensor("xs", [C, NQ], BF16, kind="ExternalInput")
    yf = nc.dram_tensor("yf", [C, N], BF16, kind="ExternalInput")
    o_v = nc.dram_tensor("o_v", [TILES, 128, G * 8], BF16, kind="ExternalOutput")
    o_i = nc.dram_tensor("o_i", [TILES, 128, G * 8], U16, kind="ExternalOutput")

    with TileContext(nc) as tc:
        with (
            tc.tile_pool(name="persist", bufs=1) as persist,
            tc.tile_pool(name="spool", bufs=8) as spool,
            tc.tile_pool(name="cpool", bufs=3) as cpool,
            tc.tile_pool(name="mpsum", bufs=4, space="PSUM") as mpsum,
        ):
            # separate tiles per chunk so the first matmuls depend only on
            # their own chunk's DMA, not the whole 3MB input load
            yc = [persist.tile([C, 1024], BF16, name=f"yc{j}", tag=f"yc{j}")
                  for j in range(N // 1024)]
            xc = [persist.tile([C, 512], BF16, name=f"xc{j}", tag=f"xc{j}")
                  for j in range(NQ // 512)]
            nc.sync.dma_start(yc[0], yf[:, :1024])
            nc.sync.dma_start(xc[0], xs[:, :512])
            nc.sync.dma_start(yc[1], yf[:, 1024:2048])
            for j in range(2, N // 1024):
                nc.sync.dma_start(yc[j], yf[:, j * 1024:(j + 1) * 1024])
            for j in range(1, NQ // 512):
                nc.sync.dma_start(xc[j], xs[:, j * 512:(j + 1) * 512])

            for t in range(TILES):
                lhsT = xc[t // 4][:, (t % 4) * 128:(t % 4 + 1) * 128]
                Vt = cpool.tile([128, G * 8], BF16, tag="V")
                It = cpool.tile([128, G * 8], U16, tag="I")
                for q in range(N // QT):
                    ps = mpsum.tile([128, QT], F32, tag="ps")
                    for i in range(4):
                        nc.tensor.matmul(
                            ps[:, i * GS:(i + 1) * GS], lhsT,
                            yn[:, q * QT + i * GS: q * QT + (i + 1) * GS],
                            start=True, stop=True)
                    S16 = spool.tile([128, QT], BF16, tag="S16")
                    nc.scalar.copy(S16, ps)
                    T1 = mpool.tile([128, GS], BF16, tag="T1")
                    T2 = mpool.tile([128, GS], BF16, tag="T2")
                    M4 = mpool.tile([128, GS], BF16, tag="M4")
                    nc.vector.tensor_max(T1, S16[:, 0:GS], S16[:, GS:2 * GS])
                    nc.vector.tensor_max(T2, S16[:, 2 * GS:3 * GS],
                                         S16[:, 3 * GS:4 * GS])
                    nc.vector.tensor_max(M4, T1, T2)
                    for k in range(4):
                        g = 4 * q + k
                        m4k = M4[:, k * 128:(k + 1) * 128]
                        nc.vector.max(Vt[:, 8 * g:8 * g + 8], m4k)
                        nc.vector.max_index(It[:, 8 * g:8 * g + 8],
                                            Vt[:, 8 * g:8 * g + 8], m4k)

                nc.sync.dma_start(o_v[t, :, :], Vt)
                nc.sync.dma_start(o_i[t, :, :], It)
    nc.finalize()
    return nc


def _host_normalize(t):
    # mimics reference._l2_normalize over axis 0 of a [C, N] f32 array
    n = np.sqrt(np.sum(t * t, axis=0, keepdims=True, dtype=np.float32),
                dtype=np.float32)
    return (t / np.maximum(n, np.float32(EPS))).astype(np.float32)


def kernel(x, y):
    x = np.ascontiguousarray(np.asarray(x, dtype=np.float32)[..., 0])  # (B, C, N)
    y = np.ascontiguousarray(np.asarray(y, dtype=np.float32)[..., 0])

    xn = np.stack([_host_normalize(x[b]) for b in range(B)])
    yn = np.stack([_host_normalize(y[b]) for b in range(B)])

    if "nc" not in _CACHED:
        _CACHED["nc"] = _build()
    nc = _CACHED["nc"]

    in_maps = []
    for k in range(8):
        b, h = k // 2, k % 2
        in_maps.append({
            "xs": np.ascontiguousarray(
                xn[b, :, h * NQ:(h + 1) * NQ]).astype(ml_dtypes.bfloat16),
            "yf": yn[b].astype(ml_dtypes.bfloat16),
        })

    trace = bool(int(os.environ.get("KNN_TRACE", "0")))
    res = run_bass_kernel_spmd(nc, in_maps, core_ids=list(range(8)), trace=trace)
    if res.exec_time_ns is not None:
        print(f"HW exec time: {res.exec_time_ns} ns")
        _CACHED["exec_time_ns"] = res.exec_time_ns

    nn_idx = np.zeros((B, N, KK), np.int32)
    need_fallback = []
    diag_max = 0.0
    # slot layout: kept slot (g, m): q = g//4, k = g%4, j = 128*k + m,
    # original candidates = 2048*q + j + 512*i, i in 0..3
    slot_g = np.arange(G * 8, dtype=np.int64) >> 3              # [128]
    slot_qbase = (slot_g // 4) * QT
    slot_kbase = (slot_g % 4) * 128
    for kcore in range(8):
        b, h = kcore // 2, kcore % 2
        vv = res.results[kcore]["o_v"].reshape(NQ, G * 8).astype(np.float64)
        mm = res.results[kcore]["o_i"].reshape(NQ, G * 8).astype(np.int64)
        j = slot_kbase[None, :] + mm                            # [NQ, 128]
        base = slot_qbase[None, :] + j                          # [NQ, 128]

        # top T_SLOTS by slot value
        order = np.argsort(-vv, axis=1, kind="stable")[:, :T_SLOTS]
        rows = np.arange(NQ)[:, None]
        cbase = base[rows, order]                               # [NQ, T]
        cand = (cbase[:, :, None] +
                np.arange(0, 4 * GS, GS)[None, None, :]).reshape(NQ, -1)

        # exact scores fp64, in two row-chunks to bound memory
        ynbT = np.ascontiguousarray(yn[b].T)                    # [N, C]
        xh = xn[b][:, h * NQ:(h + 1) * NQ]                      # [C, NQ]
        s_ex = np.empty(cand.shape, np.float64)
        for lo in range(0, NQ, 1024):
            hi = lo + 1024
            gsel = ynbT[cand[lo:hi]].astype(np.float64)         # [ch, 4T, C]
            s_ex[lo:hi] = np.matmul(
                gsel, xh.T[lo:hi].astype(np.float64)[:, :, None])[..., 0]

        # exact stable top-KK among refined
        sel = np.lexsort((cand, -s_ex), axis=1)[:, :KK]
        top_idx = np.take_along_axis(cand, sel, axis=1)
        top_s = np.take_along_axis(s_ex, sel, axis=1)
        nn_idx[b, h * NQ:(h + 1) * NQ, :] = top_idx

        # diagnostic: how far can exact scores sit above the kept slot value
        vmax = np.take_along_axis(vv, order, axis=1)            # [NQ, T]
        diag_max = max(diag_max, float(
            (s_ex - np.repeat(vmax, 2, axis=1)).max()))

        # safety: hidden candidate h has bf16(s_h) <= v8_g (its group's 8th
        # kept slot value) or <= v_(T+1) (slot kept but not refined), so
        # exact(h) <= v*(1+2^-8) + EPS_MM. flag row if that can reach the
        # 33rd exact score.
        cutoff = top_s[:, 32]
        v8 = vv[:, 7::8]                                        # [NQ, G]
        bnd_g = v8 + np.abs(v8) * (2.0 ** -8) + EPS_MM
        risk_g = (bnd_g >= cutoff[:, None]).any(axis=1)
        vT = np.sort(vv, axis=1)[:, ::-1][:, T_SLOTS]
        risk_T = (vT + np.abs(vT) * (2.0 ** -8) + EPS_MM) >= cutoff
        risk = risk_g | risk_T
        for r in np.nonzero(risk)[0]:
            need_fallback.append((b, h * NQ + int(r)))

    if need_fallback:
        by_batch = {}
        for b, n_ in need_fallback:
            by_batch.setdefault(b, []).append(n_)
        for b, rows_ in by_batch.items():
            ynb = yn[b].astype(np.float64)                      # (C, N)
            xnr = xn[b][:, rows_].astype(np.float64)            # (C, R)
            s = xnr.T @ ynb                                     # (R, N)
            part = np.argpartition(-s, KK + 8, axis=1)[:, :KK + 8]
            rr = np.arange(len(rows_))[:, None]
            pvx = -s[rr, part]
            order = np.lexsort((part, pvx), axis=1)[:, :KK]
            top = np.take_along_axis(part, order, axis=1)
            nn_idx[b, rows_, :] = top

    _CACHED["fallback_rows"] = len(need_fallback)
    _CACHED["diag_max"] = diag_max

    center = np.broadcast_to(np.arange(N, dtype=np.int32)[None, :, None],
                             (B, N, K_OUT))
    edge = np.stack([np.ascontiguousarray(nn_idx[:, :, ::DIL]), center], axis=0)
    return edge.astype(np.int32)
